# revision 38
# baseline (speedup 1.0000x reference)
"""GAT encoder on 8 TRN2 NeuronCores via Bass/Tile.

Sharding: nodes (and incident edges, partitioned by destination) across cores.
Per layer: per-edge src features are gathered from a replicated node-feature
table in DRAM via one dma_gather per edge; the dst-side attention term is
computed without a gather, via a static transposed one-hot mask (streamed
from DRAM) matmul'd with per-destination-window al_dst vectors.
Segment-softmax + scatter-add are one-hot matmuls on the tensor engine
(edges grouped into 128-node destination windows); BatchNorm stats and the
final attention pooling use AllReduce; the layer-2 message table is built
with an AllGather.
"""

import sys

sys.path.insert(0, "/opt/trn_rl_repo")

import numpy as np
import ml_dtypes

import concourse.bass as bass
import concourse.bacc as bacc
import concourse.tile as tile
import concourse.mybir as mybir

BF16 = ml_dtypes.bfloat16
FP32 = mybir.dt.float32
MBF16 = mybir.dt.bfloat16
I16 = mybir.dt.int16
AX = mybir.AxisListType
ALU = mybir.AluOpType
ACTF = mybir.ActivationFunctionType

P = 128
CHT = 16  # edge tiles per gather chunk (2048 edges)
NEG = 0.2
EPS = 1e-5


# ---------------------------------------------------------------- host prep
def prep(inputs, ncores, GB=64):
    x = np.asarray(inputs["x"], np.float32)
    ea = np.asarray(inputs["edge_attr"], np.float32)
    ei = np.asarray(inputs["edge_index"], np.int64)
    batch = np.asarray(inputs["batch"], np.int64)

    N, F_IN = x.shape
    E, ED = ea.shape
    H1, HID = 4, 64
    F1 = H1 * HID  # 256
    assert N % ncores == 0
    NL = N // ncores
    NBLK = (NL + P - 1) // P
    NLP = NBLK * P
    NPG = ((N + P - 1) // P) * P  # padded global nodes

    src = ei[0].astype(np.int64)
    dst = ei[1].astype(np.int64)

    # self loops with fill_value='mean' edge_attr
    cnt = np.bincount(dst, minlength=N).astype(np.float32)
    sea = np.zeros((N, ED), np.float32)
    np.add.at(sea, dst, ea)
    mean_ea = sea / np.maximum(cnt, 1.0)[:, None]
    src_all = np.concatenate([src, np.arange(N)])
    dst_all = np.concatenate([dst, np.arange(N)])
    ea_all = np.concatenate([ea, mean_ea], axis=0)

    core_of = dst_all // NL
    win_of = (dst_all - core_of * NL) // P
    order = np.lexsort((win_of, core_of))
    so_src, so_dst, so_core, so_win = (
        src_all[order],
        dst_all[order],
        core_of[order],
        win_of[order],
    )
    so_ea = ea_all[order]

    counts = np.zeros((ncores, NBLK), np.int64)
    np.add.at(counts, (so_core, so_win), 1)
    T_w = np.maximum(1, (np.max(counts, axis=0) + P - 1) // P)  # tiles per window
    tiles_total = int(T_w.sum())
    r = (-tiles_total) % CHT
    T_w[NBLK - 1] += r
    tiles_total += r
    EPC = tiles_total * P
    CH = tiles_total // CHT

    flat_counts = counts.ravel()
    starts = np.concatenate([[0], np.cumsum(flat_counts)[:-1]]).reshape(ncores, NBLK)

    srcidx = np.zeros((ncores, EPC), np.int16)
    dstrel = np.full((ncores, EPC), -1.0, np.float32)
    ea_core = np.zeros((ncores, EPC, ED), np.float32)

    woff = np.concatenate([[0], np.cumsum(np.asarray(T_w) * P)[:-1]])
    win_of_tile = []
    for w in range(NBLK):
        win_of_tile += [w] * int(T_w[w])
    for c in range(ncores):
        for w in range(NBLK):
            k = int(counts[c, w])
            s = int(starts[c, w])
            o = int(woff[w])
            srcidx[c, o : o + k] = so_src[s : s + k]
            dstrel[c, o : o + k] = (so_dst[s : s + k] - c * NL - w * P).astype(
                np.float32
            )
            ea_core[c, o : o + k] = so_ea[s : s + k]

    # static one-hot masks:
    #   mtT[c, d, e] = 1 if dstrel[c, e] == d   (dst on partition, edge on free)
    #   mt[c, p, tile*128 + d] = 1 if dstrel[c, tile*128+p] == d  (edge on partition)
    mtT = np.zeros((ncores, P, EPC), np.float32)
    mtE = np.zeros((ncores, P, EPC), np.float32)
    for c in range(ncores):
        dr = dstrel[c]
        valid = dr >= 0
        e_idx = np.nonzero(valid)[0]
        d_idx = dr[valid].astype(np.int64)
        mtT[c, d_idx, e_idx] = 1.0
        mtE[c, e_idx % P, (e_idx // P) * P + d_idx] = 1.0

    # weight folds
    W1 = np.asarray(inputs["W1"], np.float32)
    We1 = np.asarray(inputs["We1"], np.float32)
    as1 = np.asarray(inputs["att_src1"], np.float32)
    ad1 = np.asarray(inputs["att_dst1"], np.float32)
    ae1 = np.asarray(inputs["att_edge1"], np.float32)
    W2 = np.asarray(inputs["W2"], np.float32)
    We2 = np.asarray(inputs["We2"], np.float32)
    as2 = np.asarray(inputs["att_src2"], np.float32)
    ad2 = np.asarray(inputs["att_dst2"], np.float32)
    ae2 = np.asarray(inputs["att_edge2"], np.float32)

    def fold(W, a, H):
        return np.einsum("fhk,hk->fh", W.reshape(W.shape[0], H, HID), a)

    ws1, wd1, Ae1 = fold(W1, as1, H1), fold(W1, ad1, H1), fold(We1, ae1, H1)
    ws2, wd2, Ae2 = fold(W2, as2, 1), fold(W2, ad2, 1), fold(We2, ae2, 1)

    rhs1 = np.concatenate([W1, ws1], axis=1)  # [F_IN, 260]
    w2comb = np.concatenate([W2, wd2, ws2], axis=1)  # [F1, 66]: [xs2 | ald2 | als2]

    xT = np.zeros((F_IN, NPG), np.float32)
    xT[:, :N] = x.T

    xlocT = np.zeros((ncores, F_IN, NLP), np.float32)
    for c in range(ncores):
        xlocT[c, :, :NL] = x[c * NL : (c + 1) * NL].T

    batchrel = np.full((ncores, P, NBLK), -1.0, np.float32)
    for c in range(ncores):
        ids = np.arange(NL) + c * NL
        b = batch[ids].astype(np.float32)
        batchrel[c, :, :] = (
            np.pad(b, (0, NLP - NL), constant_values=-1.0).reshape(NBLK, P).T
        )

    g1 = np.asarray(inputs["g1"], np.float32)
    b1 = np.asarray(inputs["b1"], np.float32)
    g2 = np.asarray(inputs["g2"], np.float32)
    b2 = np.asarray(inputs["b2"], np.float32)
    Wg = np.asarray(inputs["Wg"], np.float32)  # [HID, 1]

    def wrap_idx(a):  # [EPC] -> [128, EPC//16] (16-wrap replicated x8)
        return np.ascontiguousarray(np.tile(a.reshape(-1, 16).T, (8, 1)))

    def tile128(a):  # [EPC] -> [128, EPC//128]
        return np.ascontiguousarray(a.reshape(-1, P).T)

    meta = dict(
        N=N, F_IN=F_IN, E=E, ED=ED, GB=GB, H1=H1, HID=HID, F1=F1,
        NL=NL, NBLK=NBLK, NLP=NLP, NPG=NPG, EPC=EPC, CH=CH,
        T_w=[int(t) for t in T_w], win_of_tile=win_of_tile, ncores=ncores,
    )

    shared = {
        "xT": xT.astype(BF16),
        "rhs1": rhs1.astype(BF16),
        "wd1": wd1.astype(BF16),
        "w2comb": w2comb.astype(np.float32),
        "Ae1": Ae1.astype(BF16),
        "Ae2": Ae2.astype(BF16),
        "iota128": np.ascontiguousarray(
            np.broadcast_to(np.arange(P, dtype=np.float32), (P, P))
        ).astype(BF16),
        "iotaGB": np.ascontiguousarray(
            np.broadcast_to(np.arange(GB, dtype=np.float32), (P, GB))
        ),
        "ident": np.eye(P, dtype=np.float32),
        "g1r": g1.reshape(1, F1).copy(),
        "b1r": b1.reshape(1, F1).copy(),
        "g2r": g2.reshape(1, HID).copy(),
        "b2r": b2.reshape(1, HID).copy(),
        "WgF": np.ascontiguousarray(np.broadcast_to(Wg[:, 0], (P, HID))),
    }
    percore = []
    for c in range(ncores):
        percore.append(
            {
                "srcidx": wrap_idx(srcidx[c]),
                "dstrel": tile128(dstrel[c]),
                "eaT": np.ascontiguousarray(ea_core[c].T).astype(BF16),  # [ED, EPC]
                "mtT": mtT[c].astype(BF16),  # [128, EPC]
                "mtE": mtE[c].astype(BF16),  # [128, EPC]
                "xlocT": xlocT[c].astype(BF16),  # [F_IN, NLP]
                "batchrel": np.ascontiguousarray(batchrel[c]),
            }
        )
    return meta, shared, percore


# ---------------------------------------------------------------- builder
def build(meta, dbg=False):
    N, F_IN, ED = meta["N"], meta["F_IN"], meta["ED"]
    GB, H1, HID, F1 = meta["GB"], meta["H1"], meta["HID"], meta["F1"]
    NL, NBLK, NLP, NPG = meta["NL"], meta["NBLK"], meta["NLP"], meta["NPG"]
    EPC, CH, T_w = meta["EPC"], meta["CH"], meta["T_w"]
    win_of_tile = meta["win_of_tile"]
    ncores = meta["ncores"]
    NT_G = NPG // P
    AW1 = F1 + H1  # 260: [xs | al_src]
    ROW1 = 384  # table1 bf16 cols (768B rows)
    ROW2 = 128  # table2 bf16 cols (256B rows)
    AW2 = HID + 2  # 66: [xs2 | als2 | ald2]
    tiles_total = EPC // P
    TINY = 1e-30
    BT = 8  # phase-A tiles per DMA batch

    nc = bacc.Bacc(None, target_bir_lowering=False, debug=False)

    d_xT = nc.dram_tensor("xT", [F_IN, NPG], MBF16, kind="ExternalInput")
    d_rhs1 = nc.dram_tensor("rhs1", [F_IN, AW1], MBF16, kind="ExternalInput")
    d_wd1 = nc.dram_tensor("wd1", [F_IN, H1], MBF16, kind="ExternalInput")
    d_w2comb = nc.dram_tensor("w2comb", [F1, AW2], FP32, kind="ExternalInput")
    d_Ae1 = nc.dram_tensor("Ae1", [ED, H1], MBF16, kind="ExternalInput")
    d_Ae2 = nc.dram_tensor("Ae2", [ED, 1], MBF16, kind="ExternalInput")
    d_iota = nc.dram_tensor("iota128", [P, P], MBF16, kind="ExternalInput")
    d_iotaG = nc.dram_tensor("iotaGB", [P, GB], FP32, kind="ExternalInput")
    d_ident = nc.dram_tensor("ident", [P, P], FP32, kind="ExternalInput")
    d_g1 = nc.dram_tensor("g1r", [1, F1], FP32, kind="ExternalInput")
    d_b1 = nc.dram_tensor("b1r", [1, F1], FP32, kind="ExternalInput")
    d_g2 = nc.dram_tensor("g2r", [1, HID], FP32, kind="ExternalInput")
    d_b2 = nc.dram_tensor("b2r", [1, HID], FP32, kind="ExternalInput")
    d_WgF = nc.dram_tensor("WgF", [P, HID], FP32, kind="ExternalInput")
    d_srci = nc.dram_tensor("srcidx", [P, EPC // 16], I16, kind="ExternalInput")
    d_dstrel = nc.dram_tensor("dstrel", [P, tiles_total], FP32, kind="ExternalInput")
    d_eaT = nc.dram_tensor("eaT", [ED, EPC], MBF16, kind="ExternalInput")
    d_mtT = nc.dram_tensor("mtT", [P, EPC], MBF16, kind="ExternalInput")
    d_mtE = nc.dram_tensor("mtE", [P, EPC], MBF16, kind="ExternalInput")
    d_xlocT = nc.dram_tensor("xlocT", [F_IN, NLP], MBF16, kind="ExternalInput")
    d_brel = nc.dram_tensor("batchrel", [P, NBLK], FP32, kind="ExternalInput")
    d_out = nc.dram_tensor("out", [GB, HID], FP32, kind="ExternalOutput")
    if dbg:
        d_dbg1 = nc.dram_tensor("dbg_out1", [P, NBLK * F1], MBF16, kind="ExternalOutput")
        d_dbgh2 = nc.dram_tensor("dbg_h2", [P, NBLK * HID], MBF16, kind="ExternalOutput")

    rg = [list(range(ncores))]

    with tile.TileContext(nc) as tc:
        with (
            tc.tile_pool(name="const", bufs=1) as cpool,
            tc.tile_pool(name="big", bufs=1) as bigpool,
            tc.tile_pool(name="stg", bufs=2) as stgA,
            tc.tile_pool(name="smal", bufs=2) as spool,
            tc.tile_pool(name="dram", bufs=1, space="DRAM") as dram,
        ):
            # ---- constants to SBUF
            def cload(shape, dt, src, nm):
                t = cpool.tile(shape, dt, tag=nm, name=nm)
                nc.sync.dma_start(t[:], src[:])
                return t

            c_rhs1 = cload([F_IN, AW1], MBF16, d_rhs1, "c_rhs1")
            c_wd1 = cload([F_IN, H1], MBF16, d_wd1, "c_wd1")
            c_w2 = cpool.tile([P, 2, AW2], FP32)
            nc.sync.dma_start(c_w2[:, 0, :], d_w2comb[0:P, :])
            nc.sync.dma_start(c_w2[:, 1, :], d_w2comb[P:F1, :])
            c_Ae1 = cload([ED, H1], MBF16, d_Ae1, "c_Ae1")
            c_Ae2 = cload([ED, 1], MBF16, d_Ae2, "c_Ae2")
            c_iota = cload([P, P], MBF16, d_iota, "c_iota")
            c_iotaG = cload([P, GB], FP32, d_iotaG, "c_iotaG")
            c_ident = cload([P, P], FP32, d_ident, "c_ident")
            c_g1 = cload([1, F1], FP32, d_g1, "c_g1")
            c_b1 = cload([1, F1], FP32, d_b1, "c_b1")
            c_g2 = cload([1, HID], FP32, d_g2, "c_g2")
            c_b2 = cload([1, HID], FP32, d_b2, "c_b2")
            c_WgF = cload([P, HID], FP32, d_WgF, "c_WgF")
            c_srci = cload([P, EPC // 16], I16, d_srci, "c_srci")
            c_dstrel = cload([P, tiles_total], FP32, d_dstrel, "c_dstrel")
            c_xloc = cload([F_IN, NLP], MBF16, d_xlocT, "c_xloc")
            c_brel = cload([P, NBLK], FP32, d_brel, "c_brel")
            c_ones = cpool.tile([P, 1], FP32)
            nc.gpsimd.memset(c_ones[:], 1.0)
            c_ones1 = cpool.tile([1, P], FP32)
            nc.gpsimd.memset(c_ones1[:], 1.0)
            c_onesb = cpool.tile([P, 1], MBF16)
            nc.gpsimd.memset(c_onesb[:], 1.0)
            c_identb = cpool.tile([P, P], MBF16)
            nc.vector.tensor_copy(c_identb[:], c_ident[:])
            c_aldw = cpool.tile([P, NBLK * H1], MBF16, tag="c_aldw", name="c_aldw")
            c_aldw2 = cpool.tile([P, NBLK], MBF16, tag="c_aldw2", name="c_aldw2")

            table1 = dram.tile([NPG, ROW1], MBF16)
            table2 = dram.tile([N, ROW2], MBF16, addr_space="Shared")
            ag_in = dram.tile([NL, ROW2], MBF16)

            h2 = bigpool.tile([P, NBLK * HID], MBF16, tag="h2")

            # ================= Layer 1 + layer-2 table =================
            with (
                tc.tile_pool(name="big1", bufs=1) as big1,
                tc.tile_pool(name="gath1", bufs=2) as gpool,
                tc.tile_pool(name="mbuf1", bufs=2) as mpool,
                tc.tile_pool(name="alph1", bufs=2) as apool,
            ):
                out1 = big1.tile([P, NBLK * F1], MBF16, tag="out1")

                with tc.tile_pool(name="psA", bufs=6, space="PSUM") as psA:
                    # Phase A: node table (replicated compute over all nodes)
                    stgs = []
                    for i in range(2):
                        s_ = stgA.tile(
                            [P, BT, ROW1], MBF16, tag=f"stgm_{i}", name=f"stgm_{i}"
                        )
                        nc.vector.memset(s_[:, :, AW1:ROW1], 0.0)
                        stgs.append(s_)
                    bi = 0
                    for b0 in range(0, NT_G, BT):
                        nb = min(BT, NT_G - b0)
                        xt = stgA.tile([P, BT * P], MBF16, tag="xt")
                        nc.sync.dma_start(
                            xt[:, 0 : nb * P], d_xT[:, b0 * P : (b0 + nb) * P]
                        )
                        stg = stgs[bi % 2]
                        bi += 1
                        for k in range(nb):
                            ps = psA.tile([P, AW1], FP32, tag="psA")
                            nc.tensor.matmul(
                                ps[:], xt[:, k * P : (k + 1) * P], c_rhs1[:],
                                start=True, stop=True,
                            )
                            if k % 2 == 0:
                                nc.scalar.activation(stg[:, k, 0:AW1], ps[:], ACTF.Copy)
                            else:
                                nc.vector.tensor_copy(stg[:, k, 0:AW1], ps[:])
                        nc.sync.dma_start(
                            table1[b0 * P : (b0 + nb) * P, :].rearrange(
                                "(k p) c -> p k c", p=P
                            ),
                            stg[:, 0:nb, :],
                        )

                    # per-window al_dst vectors from local x
                    for w in range(NBLK):
                        psd = psA.tile([P, H1], FP32, tag="psA")
                        nc.tensor.matmul(
                            psd[:], c_xloc[:, w * P : (w + 1) * P], c_wd1[:],
                            start=True, stop=True,
                        )
                        nc.scalar.activation(
                            c_aldw[:, w * H1 : (w + 1) * H1], psd[:], ACTF.Copy
                        )

                with (
                    tc.tile_pool(name="psAle1", bufs=2, space="PSUM") as psAle,
                    tc.tile_pool(name="psAgg1", bufs=3, space="PSUM") as psAgg,
                    tc.tile_pool(name="psS1", bufs=1, space="PSUM") as psS,
                ):
                    chunk_bufs = {}

                    def emit_chunk1(ch):
                        e0 = ch * CHT * P
                        eat = gpool.tile([ED, CHT * P], MBF16, tag="ea")
                        nc.sync.dma_start(eat[:], d_eaT[:, e0 : e0 + CHT * P])
                        mtt = gpool.tile([P, CHT * P], MBF16, tag="mtT")
                        nc.sync.dma_start(mtt[:], d_mtT[:, e0 : e0 + CHT * P])
                        mts = gpool.tile(
                            [P, CHT * P], MBF16, tag=f"mtE{ch % 2}"
                        )
                        nc.sync.dma_start(mts[:], d_mtE[:, e0 : e0 + CHT * P])
                        g1t = gpool.tile([P, CHT, ROW1], MBF16, tag=f"g1{ch % 2}")
                        nc.gpsimd.dma_gather(
                            g1t[:, :, :], table1[:, :],
                            c_srci[:, ch * P : (ch + 1) * P],
                            CHT * P, CHT * P, ROW1, single_packet=False,
                        )
                        pale = psAle.tile([P, CHT, H1], FP32, tag="pale")
                        for t in range(CHT):
                            w = win_of_tile[ch * CHT + t]
                            nc.tensor.matmul(
                                pale[:, t, :], eat[:, t * P : (t + 1) * P], c_Ae1[:],
                                start=True, stop=False,
                            )
                            nc.tensor.matmul(
                                pale[:, t, :], mtt[:, t * P : (t + 1) * P],
                                c_aldw[:, w * H1 : (w + 1) * H1],
                                start=False, stop=True,
                            )
                        alpha = apool.tile([P, CHT, H1], FP32, tag="alpha")
                        nc.vector.tensor_tensor(
                            alpha[:], g1t[:, :, F1 : F1 + H1], pale[:], ALU.add
                        )
                        lr = apool.tile([P, CHT, H1], FP32, tag="lr1")
                        nc.vector.tensor_scalar(
                            lr[:], alpha[:], 0.0, 1.0 - NEG, ALU.max, ALU.mult
                        )
                        nc.vector.scalar_tensor_tensor(
                            alpha[:], alpha[:], NEG, lr[:], ALU.mult, ALU.add
                        )
                        msgw = mpool.tile([P, CHT, F1 + H1], MBF16, tag="msgw")
                        nc.scalar.activation(msgw[:, :, F1 : F1 + H1], alpha[:], ACTF.Exp)
                        nc.vector.tensor_tensor(
                            msgw[:, :, 0:F1].rearrange("p t (h f) -> p t h f", f=HID),
                            g1t[:, :, 0:F1].rearrange("p t (h f) -> p t h f", f=HID),
                            msgw[:, :, F1 : F1 + H1]
                            .unsqueeze(3)
                            .broadcast_to((P, CHT, H1, HID)),
                            ALU.mult,
                        )
                        chunk_bufs[ch] = (msgw, mts)

                    psS1 = psS.tile([1, F1], FP32, tag="psS1")
                    psS2 = psS.tile([1, F1], FP32, tag="psS2")

                    def stats1(w):
                        nc.tensor.matmul(
                            psS1[:], c_onesb[:], out1[:, w * F1 : (w + 1) * F1],
                            start=(w == 0), stop=(w == NBLK - 1),
                        )
                        sqw = spool.tile([P, F1], MBF16, tag="sqw1")
                        nc.scalar.activation(
                            sqw[:], out1[:, w * F1 : (w + 1) * F1], ACTF.Square
                        )
                        nc.tensor.matmul(
                            psS2[:], c_onesb[:], sqw[:],
                            start=(w == 0), stop=(w == NBLK - 1),
                        )

                    t = 0
                    for w in range(NBLK):
                        psW = psAgg.tile([P, F1 + H1], FP32, tag="aggW")
                        for j in range(T_w[w]):
                            ch, tt = t // CHT, t % CHT
                            if tt == 0:
                                emit_chunk1(ch)
                            msgw, mts = chunk_bufs[ch]
                            nc.tensor.matmul(
                                psW[:], mts[:, tt * P : (tt + 1) * P], msgw[:, tt, :],
                                start=(j == 0), stop=(j == T_w[w] - 1),
                            )
                            t += 1
                        rden = spool.tile([P, H1], FP32, tag="rden1")
                        nc.vector.tensor_scalar(
                            rden[:], psW[:, F1 : F1 + H1], TINY, None, ALU.max
                        )
                        nc.vector.reciprocal(rden[:], rden[:])
                        nc.vector.tensor_tensor(
                            out1[:, w * F1 : (w + 1) * F1].rearrange(
                                "p (h f) -> p h f", f=HID
                            ),
                            psW[:, 0:F1].rearrange("p (h f) -> p h f", f=HID),
                            rden[:].unsqueeze(2).broadcast_to((P, H1, HID)),
                            ALU.mult,
                        )
                        if w >= 2:
                            stats1(w - 2)
                    stats1(NBLK - 2)
                    stats1(NBLK - 1)
                    if dbg:
                        nc.sync.dma_start(d_dbg1[:], out1[:])

                    # BN1 stats allreduce
                    bn1buf = spool.tile([1, 2 * F1], FP32, tag="bn1")
                    nc.vector.tensor_copy(bn1buf[:, 0:F1], psS1[:])
                    nc.vector.tensor_copy(bn1buf[:, F1 : 2 * F1], psS2[:])
                    bn1_in = dram.tile([1, 2 * F1], FP32)
                    bn1_out = dram.tile([1, 2 * F1], FP32, addr_space="Shared")
                    nc.sync.dma_start(bn1_in[:], bn1buf[:])
                    nc.gpsimd.collective_compute(
                        "AllReduce", ALU.add, replica_groups=rg,
                        ins=[bn1_in.opt()], outs=[bn1_out.opt()],
                    )
                    bnr1 = spool.tile([1, 2 * F1], FP32, tag="bn1r")
                    nc.sync.dma_start(bnr1[:], bn1_out[:])

                # BN1 row math; apply fused into transposed layout; layer-2 table
                with tc.tile_pool(name="psDE", bufs=2, space="PSUM") as psDE:
                    mean1 = spool.tile([1, F1], FP32, tag="mean1")
                    nc.scalar.activation(
                        mean1[:], bnr1[:, 0:F1], ACTF.Copy, scale=1.0 / N
                    )
                    var1 = spool.tile([1, F1], FP32, tag="var1")
                    nc.scalar.activation(
                        var1[:], bnr1[:, F1 : 2 * F1], ACTF.Copy, scale=1.0 / N
                    )
                    msq1 = spool.tile([1, F1], FP32, tag="msq1")
                    nc.vector.tensor_tensor(msq1[:], mean1[:], mean1[:], ALU.mult)
                    nc.vector.tensor_tensor(var1[:], var1[:], msq1[:], ALU.subtract)
                    nc.vector.tensor_scalar(var1[:], var1[:], EPS, None, ALU.add)
                    std1 = spool.tile([1, F1], FP32, tag="std1")
                    nc.scalar.activation(std1[:], var1[:], ACTF.Sqrt)
                    nc.vector.reciprocal(std1[:], std1[:])
                    scl1r = spool.tile([1, F1], FP32, tag="scl1r")
                    nc.vector.tensor_tensor(scl1r[:], c_g1[:], std1[:], ALU.mult)
                    sht1r = spool.tile([1, F1], FP32, tag="sht1r")
                    nc.vector.tensor_tensor(sht1r[:], mean1[:], scl1r[:], ALU.mult)
                    nc.vector.tensor_tensor(sht1r[:], c_b1[:], sht1r[:], ALU.subtract)
                    # transpose scale/shift rows into per-partition columns
                    c_w2b = spool.tile([P, 2, AW2], MBF16, tag="c_w2b")
                    nc.vector.tensor_copy(c_w2b[:], c_w2[:])
                    sclT = spool.tile([P, 2], FP32, tag="sclT")
                    shtT = spool.tile([P, 2], FP32, tag="shtT")
                    for cc in range(2):
                        pt1 = psDE.tile([P, 2], FP32, tag="pt1")
                        nc.tensor.matmul(
                            pt1[:, 0:1], scl1r[:, cc * P : (cc + 1) * P],
                            c_ones1[:, 0:1], start=True, stop=True,
                        )
                        nc.tensor.matmul(
                            pt1[:, 1:2], sht1r[:, cc * P : (cc + 1) * P],
                            c_ones1[:, 0:1], start=True, stop=True,
                        )
                        nc.vector.tensor_copy(sclT[:, cc : cc + 1], pt1[:, 0:1])
                        nc.vector.tensor_copy(shtT[:, cc : cc + 1], pt1[:, 1:2])

                    # table2 row: [xs2 (0:64) | ald2 (64) | als2 (65) | 1.0 (66) | 0...]
                    stg2s = []
                    for i in range(3):
                        s_ = stgA.tile(
                            [P, ROW2], MBF16, tag=f"stg2m_{i}", name=f"stg2m_{i}"
                        )
                        nc.vector.memset(s_[:, AW2 + 1 : ROW2], 0.0)
                        nc.vector.memset(s_[:, AW2 : AW2 + 1], 1.0)
                        stg2s.append(s_)
                    for w in range(NBLK):
                        hTw = spool.tile([P, 2, P], MBF16, tag="hTw")
                        for cc in range(2):
                            psT = psDE.tile([P, P], MBF16, tag="psT")
                            nc.tensor.transpose(
                                psT[:],
                                out1[:, w * F1 + cc * P : w * F1 + (cc + 1) * P],
                                c_identb[:],
                            )
                            nc.scalar.activation(
                                hTw[:, cc, :], psT[:], ACTF.Relu,
                                bias=shtT[:, cc : cc + 1], scale=sclT[:, cc : cc + 1],
                            )
                        psX = psDE.tile([P, AW2], FP32, tag="psX")
                        nc.tensor.matmul(
                            psX[:], hTw[:, 0, :], c_w2b[:, 0, :],
                            start=True, stop=False,
                        )
                        nc.tensor.matmul(
                            psX[:], hTw[:, 1, :], c_w2b[:, 1, :],
                            start=False, stop=True,
                        )
                        stg2 = stg2s[w % 3]
                        nc.vector.tensor_copy(stg2[:, 0:AW2], psX[:, 0:AW2])
                        nc.vector.tensor_copy(
                            c_aldw2[:, w : w + 1], psX[:, HID : HID + 1]
                        )
                        r0, r1 = w * P, min(NL, (w + 1) * P)
                        if r1 > r0:
                            nc.sync.dma_start(ag_in[r0:r1, :], stg2[0 : r1 - r0, :])
                    nc.gpsimd.collective_compute(
                        "AllGather", ALU.bypass, replica_groups=rg,
                        ins=[ag_in.opt()], outs=[table2.opt()],
                    )

            # ================= Phase F: layer-2 edges ======================
            # table2 row: [xs2 (0:64) | ald2 (64) | als2 (65) | 1.0 (66) | 0...]
            with (
                tc.tile_pool(name="gath2", bufs=2) as gpool2,
                tc.tile_pool(name="alph2", bufs=2) as apool2,
                tc.tile_pool(name="psAle2", bufs=2, space="PSUM") as psAle2,
                tc.tile_pool(name="psAgg2", bufs=4, space="PSUM") as psAgg2,
                tc.tile_pool(name="psBn2", bufs=1, space="PSUM") as psBn2,
            ):
                ps_bn2a = psBn2.tile([1, HID], FP32, tag="psbn2a")
                ps_bn2b = psBn2.tile([1, HID], FP32, tag="psbn2b")
                chunk2_bufs = {}
                AWD = AW2 + 1  # 67: [xs2 | ald2 | als2 | 1.0]

                def emit_chunk2(ch):
                    e0 = ch * CHT * P
                    eat = gpool2.tile([ED, CHT * P], MBF16, tag="ea2")
                    nc.sync.dma_start(eat[:], d_eaT[:, e0 : e0 + CHT * P])
                    mtt = gpool2.tile([P, CHT * P], MBF16, tag="mtT2")
                    nc.sync.dma_start(mtt[:], d_mtT[:, e0 : e0 + CHT * P])
                    mts = gpool2.tile(
                        [P, CHT * P], MBF16, tag=f"mtE2{ch % 3}"
                    )
                    nc.sync.dma_start(mts[:], d_mtE[:, e0 : e0 + CHT * P])
                    g3t = gpool2.tile([P, CHT, ROW2], MBF16, tag=f"g3{ch % 3}")
                    nc.gpsimd.dma_gather(
                        g3t[:, :, :], table2[:, :],
                        c_srci[:, ch * P : (ch + 1) * P],
                        CHT * P, CHT * P, ROW2, single_packet=False,
                    )
                    pale = psAle2.tile([P, CHT, 1], FP32, tag="pale2")
                    for t in range(CHT):
                        w = win_of_tile[ch * CHT + t]
                        nc.tensor.matmul(
                            pale[:, t, :], eat[:, t * P : (t + 1) * P], c_Ae2[:],
                            start=True, stop=False,
                        )
                        nc.tensor.matmul(
                            pale[:, t, :], mtt[:, t * P : (t + 1) * P],
                            c_aldw2[:, w : w + 1],
                            start=False, stop=True,
                        )
                    alpha = apool2.tile([P, CHT, 1], FP32, tag="alpha2")
                    nc.vector.tensor_tensor(
                        alpha[:], g3t[:, :, HID + 1 : HID + 2], pale[:], ALU.add
                    )
                    lr = apool2.tile([P, CHT, 1], FP32, tag="lr2")
                    nc.vector.tensor_scalar(
                        lr[:], alpha[:], 0.0, 1.0 - NEG, ALU.max, ALU.mult
                    )
                    nc.vector.scalar_tensor_tensor(
                        alpha[:], alpha[:], NEG, lr[:], ALU.mult, ALU.add
                    )
                    exb = apool2.tile([P, CHT, 1], FP32, tag="exb2")
                    nc.scalar.activation(exb[:], alpha[:], ACTF.Exp)
                    for t in range(CHT):
                        nc.vector.tensor_scalar(
                            g3t[:, t, 0:AWD], g3t[:, t, 0:AWD],
                            exb[:, t, :], None, ALU.mult,
                        )
                    chunk2_bufs[ch] = (g3t, mts)

                def stats2(w):
                    nc.tensor.matmul(
                        ps_bn2a[:], c_onesb[:], h2[:, w * HID : (w + 1) * HID],
                        start=(w == 0), stop=(w == NBLK - 1),
                    )
                    sqw = spool.tile([P, HID], MBF16, tag="sqw")
                    nc.scalar.activation(
                        sqw[:], h2[:, w * HID : (w + 1) * HID], ACTF.Square
                    )
                    nc.tensor.matmul(
                        ps_bn2b[:], c_onesb[:], sqw[:],
                        start=(w == 0), stop=(w == NBLK - 1),
                    )

                def evac2(w, pso):
                    rden2 = spool.tile([P, 1], FP32, tag="rden2")
                    nc.vector.tensor_scalar(
                        rden2[:], pso[:, AWD - 1 : AWD], TINY, None, ALU.max
                    )
                    nc.vector.reciprocal(rden2[:], rden2[:])
                    nc.scalar.activation(
                        h2[:, w * HID : (w + 1) * HID], pso[:, 0:HID], ACTF.Copy,
                        scale=rden2[:],
                    )

                psos = {}
                t = 0
                for w in range(NBLK):
                    pso = psAgg2.tile([P, AWD], FP32, tag="agg2")
                    psos[w] = pso
                    for j in range(T_w[w]):
                        ch, tt = t // CHT, t % CHT
                        if tt == 0:
                            emit_chunk2(ch)
                        g3t, mts = chunk2_bufs[ch]
                        nc.tensor.matmul(
                            pso[:], mts[:, tt * P : (tt + 1) * P],
                            g3t[:, tt, 0:AWD],
                            start=(j == 0), stop=(j == T_w[w] - 1),
                        )
                        t += 1
                    if w >= 1:
                        evac2(w - 1, psos.pop(w - 1))
                    if w >= 3:
                        stats2(w - 3)
                evac2(NBLK - 1, psos.pop(NBLK - 1))
                for ww in range(max(0, NBLK - 3), NBLK):
                    stats2(ww)

                if dbg:
                    nc.sync.dma_start(d_dbgh2[:], h2[:])
                bn2buf = spool.tile([1, 2 * HID], FP32, tag="bn2")
                nc.vector.tensor_copy(bn2buf[:, 0:HID], ps_bn2a[:])
                nc.vector.tensor_copy(bn2buf[:, HID : 2 * HID], ps_bn2b[:])
                bn2_in = dram.tile([1, 2 * HID], FP32)
                bn2_out = dram.tile([1, 2 * HID], FP32, addr_space="Shared")
                nc.sync.dma_start(bn2_in[:], bn2buf[:])
                nc.gpsimd.collective_compute(
                    "AllReduce", ALU.add, replica_groups=rg,
                    ins=[bn2_in.opt()], outs=[bn2_out.opt()],
                )
                bnr2 = spool.tile([1, 2, HID], FP32, tag="bn2r")
                nc.sync.dma_start(bnr2[:].rearrange("p a b -> p (a b)"), bn2_out[:])

            # ================= Phase G: BN2 + ReLU + pool ==================
            with tc.tile_pool(name="psG", bufs=2, space="PSUM") as psG:
                mean2 = spool.tile([1, HID], FP32, tag="mean2")
                nc.scalar.activation(mean2[:], bnr2[:, 0, :], ACTF.Copy, scale=1.0 / N)
                var2 = spool.tile([1, HID], FP32, tag="var2")
                nc.scalar.activation(var2[:], bnr2[:, 1, :], ACTF.Copy, scale=1.0 / N)
                msq2 = spool.tile([1, HID], FP32, tag="msq2")
                nc.vector.tensor_tensor(msq2[:], mean2[:], mean2[:], ALU.mult)
                nc.vector.tensor_tensor(var2[:], var2[:], msq2[:], ALU.subtract)
                nc.vector.tensor_scalar(var2[:], var2[:], EPS, None, ALU.add)
                std2 = spool.tile([1, HID], FP32, tag="std2")
                nc.scalar.activation(std2[:], var2[:], ACTF.Sqrt)
                nc.vector.reciprocal(std2[:], std2[:])
                scl2r = spool.tile([1, HID], FP32, tag="scl2r")
                nc.vector.tensor_tensor(scl2r[:], c_g2[:], std2[:], ALU.mult)
                sht2r = spool.tile([1, HID], FP32, tag="sht2r")
                nc.vector.tensor_tensor(sht2r[:], mean2[:], scl2r[:], ALU.mult)
                nc.vector.tensor_tensor(sht2r[:], c_b2[:], sht2r[:], ALU.subtract)
                psb = psG.tile([P, 2, HID], FP32, tag="psb2")
                nc.tensor.matmul(
                    psb[:, 0, :], c_ones1[:], scl2r[:], start=True, stop=True
                )
                nc.tensor.matmul(
                    psb[:, 1, :], c_ones1[:], sht2r[:], start=True, stop=True
                )
                sclF = spool.tile([P, HID], FP32, tag="sclF")
                nc.vector.tensor_copy(sclF[:], psb[:, 0, :])
                shtF = spool.tile([P, HID], FP32, tag="shtF")
                nc.vector.tensor_copy(shtF[:], psb[:, 1, :])

                ps_num = psG.tile([GB, HID], FP32, tag="psnum")
                ps_den = psG.tile([GB, 1], FP32, tag="psden")
                for wp in range(0, NBLK, 2):
                    kk = min(2, NBLK - wp)
                    hb = spool.tile([P, 2, HID], FP32, tag="hb")
                    h2s = h2[:, wp * HID : (wp + kk) * HID].rearrange(
                        "p (k h) -> p k h", h=HID
                    )
                    nc.vector.tensor_tensor(
                        hb[:, 0:kk, :], h2s,
                        sclF[:].unsqueeze(1).broadcast_to((P, kk, HID)), ALU.mult
                    )
                    nc.vector.tensor_tensor(
                        hb[:, 0:kk, :], hb[:, 0:kk, :],
                        shtF[:].unsqueeze(1).broadcast_to((P, kk, HID)), ALU.add
                    )
                    nc.vector.tensor_scalar(
                        hb[:, 0:kk, :], hb[:, 0:kk, :], 0.0, None, ALU.max
                    )
                    gtmp = spool.tile([P, 2, HID], FP32, tag="gtmp")
                    nc.vector.tensor_tensor(
                        gtmp[:, 0:kk, :], hb[:, 0:kk, :],
                        c_WgF[:].unsqueeze(1).broadcast_to((P, kk, HID)), ALU.mult
                    )
                    gate = spool.tile([P, 2, 1], FP32, tag="gate")
                    nc.vector.reduce_sum(gate[:, 0:kk, :], gtmp[:, 0:kk, :], AX.X)
                    ex = spool.tile([P, 2, 1], FP32, tag="exg")
                    nc.scalar.activation(ex[:, 0:kk, :], gate[:, 0:kk, :], ACTF.Exp)
                    for k in range(kk):
                        w = wp + k
                        bex = spool.tile([P, GB], FP32, tag="bex")
                        nc.vector.tensor_scalar(
                            bex[:], c_iotaG[:], c_brel[:, w : w + 1], ex[:, k, :],
                            ALU.is_equal, ALU.mult,
                        )
                        nc.tensor.matmul(
                            ps_num[:], bex[:], hb[:, k, :],
                            start=(w == 0), stop=(w == NBLK - 1),
                        )
                        nc.tensor.matmul(
                            ps_den[:], bex[:], c_ones[:],
                            start=(w == 0), stop=(w == NBLK - 1),
                        )
                poolbuf = spool.tile([GB, HID + 1], FP32, tag="poolbuf")
                nc.vector.tensor_copy(poolbuf[:, 0:HID], ps_num[:])
                nc.vector.tensor_copy(poolbuf[:, HID : HID + 1], ps_den[:])
                pool_in = dram.tile([GB, HID + 1], FP32)
                pool_out = dram.tile([GB, HID + 1], FP32, addr_space="Shared")
                nc.sync.dma_start(pool_in[:], poolbuf[:])
                nc.gpsimd.collective_compute(
                    "AllReduce", ALU.add, replica_groups=rg,
                    ins=[pool_in.opt()], outs=[pool_out.opt()],
                )
                poolr = spool.tile([GB, HID + 1], FP32, tag="poolr")
                nc.sync.dma_start(poolr[:], pool_out[:])
                dinv = spool.tile([GB, 1], FP32, tag="dinv")
                nc.vector.reciprocal(dinv[:], poolr[:, HID : HID + 1])
                res = spool.tile([GB, HID], FP32, tag="res")
                nc.vector.tensor_scalar(
                    res[:], poolr[:, 0:HID], dinv[:], None, ALU.mult
                )
                nc.sync.dma_start(d_out[:], res[:])

    nc.compile()
    return nc


# ---------------------------------------------------------------- runner
def make_in_maps(meta, shared, percore):
    return [{**shared, **pc} for pc in percore]


def run(inputs, ncores=8, trace=False, sim=False, GB=64, dbg=False):
    meta, shared, percore = prep(inputs, ncores, GB=GB)
    nc = build(meta, dbg=dbg)
    in_maps = make_in_maps(meta, shared, percore)
    if sim:
        from concourse.bass_interp import MultiCoreSim

        msim = MultiCoreSim(nc, ncores)
        for c in range(ncores):
            for k, v in in_maps[c].items():
                msim.cores[c].tensor(k)[:] = v
        msim.simulate()
        return msim.cores[0].mem_tensor("out").copy(), (msim, meta)
    from concourse.bass_utils import run_bass_kernel_spmd

    res = run_bass_kernel_spmd(nc, in_maps, core_ids=list(range(ncores)), trace=trace)
    return res.results[0]["out"], res


# ---------------------------------------------------------------- kernel API
_CACHE = {}


def _run_full(inputs, trace=False):
    meta, shared, percore = prep(inputs, 8, GB=64)
    key = (meta["EPC"], meta["N"])
    if key not in _CACHE:
        _CACHE[key] = build(meta)
    nc = _CACHE[key]
    in_maps = make_in_maps(meta, shared, percore)
    from concourse.bass_utils import run_bass_kernel_spmd

    res = run_bass_kernel_spmd(nc, in_maps, core_ids=list(range(8)), trace=trace)
    return np.asarray(res.results[0]["out"], np.float32), res


def kernel(**inputs):
    out, _ = _run_full(inputs, trace=False)
    return out


# revision 39
# speedup vs baseline: 1.0171x; 1.0171x over previous
"""GAT encoder on 8 TRN2 NeuronCores via Bass/Tile.

Sharding: nodes (and incident edges, partitioned by destination) across cores.
Per layer: per-edge src features are gathered from a replicated node-feature
table in DRAM via one dma_gather per edge; the dst-side attention term is
computed without a gather, via a static transposed one-hot mask (streamed
from DRAM) matmul'd with per-destination-window al_dst vectors.
Segment-softmax + scatter-add are one-hot matmuls on the tensor engine
(edges grouped into 128-node destination windows); BatchNorm stats and the
final attention pooling use AllReduce; the layer-2 message table is built
with an AllGather.
"""

import sys

sys.path.insert(0, "/opt/trn_rl_repo")

import numpy as np
import ml_dtypes

import concourse.bass as bass
import concourse.bacc as bacc
import concourse.tile as tile
import concourse.mybir as mybir

BF16 = ml_dtypes.bfloat16
FP32 = mybir.dt.float32
MBF16 = mybir.dt.bfloat16
I16 = mybir.dt.int16
AX = mybir.AxisListType
ALU = mybir.AluOpType
ACTF = mybir.ActivationFunctionType

P = 128
CHT = 16  # edge tiles per gather chunk (2048 edges)
NEG = 0.2
EPS = 1e-5


# ---------------------------------------------------------------- host prep
def prep(inputs, ncores, GB=64):
    x = np.asarray(inputs["x"], np.float32)
    ea = np.asarray(inputs["edge_attr"], np.float32)
    ei = np.asarray(inputs["edge_index"], np.int64)
    batch = np.asarray(inputs["batch"], np.int64)

    N, F_IN = x.shape
    E, ED = ea.shape
    H1, HID = 4, 64
    F1 = H1 * HID  # 256
    assert N % ncores == 0
    NL = N // ncores
    NBLK = (NL + P - 1) // P
    NLP = NBLK * P
    NPG = ((N + P - 1) // P) * P  # padded global nodes

    src = ei[0].astype(np.int64)
    dst = ei[1].astype(np.int64)

    # self loops with fill_value='mean' edge_attr
    cnt = np.bincount(dst, minlength=N).astype(np.float32)
    sea = np.zeros((N, ED), np.float32)
    np.add.at(sea, dst, ea)
    mean_ea = sea / np.maximum(cnt, 1.0)[:, None]
    src_all = np.concatenate([src, np.arange(N)])
    dst_all = np.concatenate([dst, np.arange(N)])
    ea_all = np.concatenate([ea, mean_ea], axis=0)

    core_of = dst_all // NL
    win_of = (dst_all - core_of * NL) // P
    order = np.lexsort((win_of, core_of))
    so_src, so_dst, so_core, so_win = (
        src_all[order],
        dst_all[order],
        core_of[order],
        win_of[order],
    )
    so_ea = ea_all[order]

    counts = np.zeros((ncores, NBLK), np.int64)
    np.add.at(counts, (so_core, so_win), 1)
    T_w = np.maximum(1, (np.max(counts, axis=0) + P - 1) // P)  # tiles per window
    tiles_total = int(T_w.sum())
    r = (-tiles_total) % CHT
    T_w[NBLK - 1] += r
    tiles_total += r
    EPC = tiles_total * P
    CH = tiles_total // CHT

    flat_counts = counts.ravel()
    starts = np.concatenate([[0], np.cumsum(flat_counts)[:-1]]).reshape(ncores, NBLK)

    srcidx = np.zeros((ncores, EPC), np.int16)
    dstrel = np.full((ncores, EPC), -1.0, np.float32)
    ea_core = np.zeros((ncores, EPC, ED), np.float32)

    woff = np.concatenate([[0], np.cumsum(np.asarray(T_w) * P)[:-1]])
    win_of_tile = []
    for w in range(NBLK):
        win_of_tile += [w] * int(T_w[w])
    for c in range(ncores):
        for w in range(NBLK):
            k = int(counts[c, w])
            s = int(starts[c, w])
            o = int(woff[w])
            srcidx[c, o : o + k] = so_src[s : s + k]
            dstrel[c, o : o + k] = (so_dst[s : s + k] - c * NL - w * P).astype(
                np.float32
            )
            ea_core[c, o : o + k] = so_ea[s : s + k]

    # static one-hot masks:
    #   mtT[c, d, e] = 1 if dstrel[c, e] == d   (dst on partition, edge on free)
    #   mt[c, p, tile*128 + d] = 1 if dstrel[c, tile*128+p] == d  (edge on partition)
    mtT = np.zeros((ncores, P, EPC), np.float32)
    mtE = np.zeros((ncores, P, EPC), np.float32)
    for c in range(ncores):
        dr = dstrel[c]
        valid = dr >= 0
        e_idx = np.nonzero(valid)[0]
        d_idx = dr[valid].astype(np.int64)
        mtT[c, d_idx, e_idx] = 1.0
        mtE[c, e_idx % P, (e_idx // P) * P + d_idx] = 1.0

    # weight folds
    W1 = np.asarray(inputs["W1"], np.float32)
    We1 = np.asarray(inputs["We1"], np.float32)
    as1 = np.asarray(inputs["att_src1"], np.float32)
    ad1 = np.asarray(inputs["att_dst1"], np.float32)
    ae1 = np.asarray(inputs["att_edge1"], np.float32)
    W2 = np.asarray(inputs["W2"], np.float32)
    We2 = np.asarray(inputs["We2"], np.float32)
    as2 = np.asarray(inputs["att_src2"], np.float32)
    ad2 = np.asarray(inputs["att_dst2"], np.float32)
    ae2 = np.asarray(inputs["att_edge2"], np.float32)

    def fold(W, a, H):
        return np.einsum("fhk,hk->fh", W.reshape(W.shape[0], H, HID), a)

    ws1, wd1, Ae1 = fold(W1, as1, H1), fold(W1, ad1, H1), fold(We1, ae1, H1)
    ws2, wd2, Ae2 = fold(W2, as2, 1), fold(W2, ad2, 1), fold(We2, ae2, 1)

    rhs1 = np.concatenate([W1, ws1], axis=1)  # [F_IN, 260]
    w2comb = np.concatenate([W2, wd2, ws2], axis=1)  # [F1, 66]: [xs2 | ald2 | als2]

    xT = np.zeros((F_IN, NPG), np.float32)
    xT[:, :N] = x.T

    xlocT = np.zeros((ncores, F_IN, NLP), np.float32)
    for c in range(ncores):
        xlocT[c, :, :NL] = x[c * NL : (c + 1) * NL].T

    batchrel = np.full((ncores, P, NBLK), -1.0, np.float32)
    for c in range(ncores):
        ids = np.arange(NL) + c * NL
        b = batch[ids].astype(np.float32)
        batchrel[c, :, :] = (
            np.pad(b, (0, NLP - NL), constant_values=-1.0).reshape(NBLK, P).T
        )

    g1 = np.asarray(inputs["g1"], np.float32)
    b1 = np.asarray(inputs["b1"], np.float32)
    g2 = np.asarray(inputs["g2"], np.float32)
    b2 = np.asarray(inputs["b2"], np.float32)
    Wg = np.asarray(inputs["Wg"], np.float32)  # [HID, 1]

    def wrap_idx(a):  # [EPC] -> [128, EPC//16] (16-wrap replicated x8)
        return np.ascontiguousarray(np.tile(a.reshape(-1, 16).T, (8, 1)))

    def tile128(a):  # [EPC] -> [128, EPC//128]
        return np.ascontiguousarray(a.reshape(-1, P).T)

    meta = dict(
        N=N, F_IN=F_IN, E=E, ED=ED, GB=GB, H1=H1, HID=HID, F1=F1,
        NL=NL, NBLK=NBLK, NLP=NLP, NPG=NPG, EPC=EPC, CH=CH,
        T_w=[int(t) for t in T_w], win_of_tile=win_of_tile, ncores=ncores,
    )

    shared = {
        "xT": xT.astype(BF16),
        "rhs1": rhs1.astype(BF16),
        "wd1": wd1.astype(BF16),
        "w2comb": w2comb.astype(np.float32),
        "Ae1": Ae1.astype(BF16),
        "Ae2": Ae2.astype(BF16),
        "iota128": np.ascontiguousarray(
            np.broadcast_to(np.arange(P, dtype=np.float32), (P, P))
        ).astype(BF16),
        "iotaGB": np.ascontiguousarray(
            np.broadcast_to(np.arange(GB, dtype=np.float32), (P, GB))
        ),
        "ident": np.eye(P, dtype=np.float32),
        "g1r": g1.reshape(1, F1).copy(),
        "b1r": b1.reshape(1, F1).copy(),
        "g2r": g2.reshape(1, HID).copy(),
        "b2r": b2.reshape(1, HID).copy(),
        "WgF": np.ascontiguousarray(np.broadcast_to(Wg[:, 0], (P, HID))),
    }
    percore = []
    for c in range(ncores):
        percore.append(
            {
                "srcidx": wrap_idx(srcidx[c]),
                "dstrel": tile128(dstrel[c]),
                "eaT": np.ascontiguousarray(ea_core[c].T).astype(BF16),  # [ED, EPC]
                "mtT": mtT[c].astype(BF16),  # [128, EPC]
                "mtE": mtE[c].astype(BF16),  # [128, EPC]
                "xlocT": xlocT[c].astype(BF16),  # [F_IN, NLP]
                "batchrel": np.ascontiguousarray(batchrel[c]),
            }
        )
    return meta, shared, percore


# ---------------------------------------------------------------- builder
def build(meta, dbg=False):
    N, F_IN, ED = meta["N"], meta["F_IN"], meta["ED"]
    GB, H1, HID, F1 = meta["GB"], meta["H1"], meta["HID"], meta["F1"]
    NL, NBLK, NLP, NPG = meta["NL"], meta["NBLK"], meta["NLP"], meta["NPG"]
    EPC, CH, T_w = meta["EPC"], meta["CH"], meta["T_w"]
    win_of_tile = meta["win_of_tile"]
    ncores = meta["ncores"]
    NT_G = NPG // P
    AW1 = F1 + H1  # 260: [xs | al_src]
    ROW1 = 384  # table1 bf16 cols (768B rows)
    ROW2 = 128  # table2 bf16 cols (256B rows)
    AW2 = HID + 2  # 66: [xs2 | als2 | ald2]
    tiles_total = EPC // P
    TINY = 1e-30
    BT = 8  # phase-A tiles per DMA batch

    nc = bacc.Bacc(None, target_bir_lowering=False, debug=False)

    d_xT = nc.dram_tensor("xT", [F_IN, NPG], MBF16, kind="ExternalInput")
    d_rhs1 = nc.dram_tensor("rhs1", [F_IN, AW1], MBF16, kind="ExternalInput")
    d_wd1 = nc.dram_tensor("wd1", [F_IN, H1], MBF16, kind="ExternalInput")
    d_w2comb = nc.dram_tensor("w2comb", [F1, AW2], FP32, kind="ExternalInput")
    d_Ae1 = nc.dram_tensor("Ae1", [ED, H1], MBF16, kind="ExternalInput")
    d_Ae2 = nc.dram_tensor("Ae2", [ED, 1], MBF16, kind="ExternalInput")
    d_iota = nc.dram_tensor("iota128", [P, P], MBF16, kind="ExternalInput")
    d_iotaG = nc.dram_tensor("iotaGB", [P, GB], FP32, kind="ExternalInput")
    d_ident = nc.dram_tensor("ident", [P, P], FP32, kind="ExternalInput")
    d_g1 = nc.dram_tensor("g1r", [1, F1], FP32, kind="ExternalInput")
    d_b1 = nc.dram_tensor("b1r", [1, F1], FP32, kind="ExternalInput")
    d_g2 = nc.dram_tensor("g2r", [1, HID], FP32, kind="ExternalInput")
    d_b2 = nc.dram_tensor("b2r", [1, HID], FP32, kind="ExternalInput")
    d_WgF = nc.dram_tensor("WgF", [P, HID], FP32, kind="ExternalInput")
    d_srci = nc.dram_tensor("srcidx", [P, EPC // 16], I16, kind="ExternalInput")
    d_dstrel = nc.dram_tensor("dstrel", [P, tiles_total], FP32, kind="ExternalInput")
    d_eaT = nc.dram_tensor("eaT", [ED, EPC], MBF16, kind="ExternalInput")
    d_mtT = nc.dram_tensor("mtT", [P, EPC], MBF16, kind="ExternalInput")
    d_mtE = nc.dram_tensor("mtE", [P, EPC], MBF16, kind="ExternalInput")
    d_xlocT = nc.dram_tensor("xlocT", [F_IN, NLP], MBF16, kind="ExternalInput")
    d_brel = nc.dram_tensor("batchrel", [P, NBLK], FP32, kind="ExternalInput")
    d_out = nc.dram_tensor("out", [GB, HID], FP32, kind="ExternalOutput")
    if dbg:
        d_dbg1 = nc.dram_tensor("dbg_out1", [P, NBLK * F1], MBF16, kind="ExternalOutput")
        d_dbgh2 = nc.dram_tensor("dbg_h2", [P, NBLK * HID], MBF16, kind="ExternalOutput")

    rg = [list(range(ncores))]

    with tile.TileContext(nc) as tc:
        with (
            tc.tile_pool(name="const", bufs=1) as cpool,
            tc.tile_pool(name="big", bufs=1) as bigpool,
            tc.tile_pool(name="stg", bufs=2) as stgA,
            tc.tile_pool(name="smal", bufs=2) as spool,
            tc.tile_pool(name="dram", bufs=1, space="DRAM") as dram,
        ):
            # ---- constants to SBUF
            def cload(shape, dt, src, nm):
                t = cpool.tile(shape, dt, tag=nm, name=nm)
                nc.sync.dma_start(t[:], src[:])
                return t

            c_rhs1 = cload([F_IN, AW1], MBF16, d_rhs1, "c_rhs1")
            c_wd1 = cload([F_IN, H1], MBF16, d_wd1, "c_wd1")
            c_w2 = cpool.tile([P, 2, AW2], FP32)
            nc.sync.dma_start(c_w2[:, 0, :], d_w2comb[0:P, :])
            nc.sync.dma_start(c_w2[:, 1, :], d_w2comb[P:F1, :])
            c_Ae1 = cload([ED, H1], MBF16, d_Ae1, "c_Ae1")
            c_Ae2 = cload([ED, 1], MBF16, d_Ae2, "c_Ae2")
            c_iota = cload([P, P], MBF16, d_iota, "c_iota")
            c_iotaG = cload([P, GB], FP32, d_iotaG, "c_iotaG")
            c_ident = cload([P, P], FP32, d_ident, "c_ident")
            c_g1 = cload([1, F1], FP32, d_g1, "c_g1")
            c_b1 = cload([1, F1], FP32, d_b1, "c_b1")
            c_g2 = cload([1, HID], FP32, d_g2, "c_g2")
            c_b2 = cload([1, HID], FP32, d_b2, "c_b2")
            c_WgF = cload([P, HID], FP32, d_WgF, "c_WgF")
            c_srci = cload([P, EPC // 16], I16, d_srci, "c_srci")
            c_dstrel = cload([P, tiles_total], FP32, d_dstrel, "c_dstrel")
            c_xloc = cload([F_IN, NLP], MBF16, d_xlocT, "c_xloc")
            c_brel = cload([P, NBLK], FP32, d_brel, "c_brel")
            c_ones = cpool.tile([P, 1], FP32)
            nc.gpsimd.memset(c_ones[:], 1.0)
            c_ones1 = cpool.tile([1, P], FP32)
            nc.gpsimd.memset(c_ones1[:], 1.0)
            c_onesb = cpool.tile([P, 1], MBF16)
            nc.gpsimd.memset(c_onesb[:], 1.0)
            c_identb = cpool.tile([P, P], MBF16)
            nc.vector.tensor_copy(c_identb[:], c_ident[:])
            c_aldw = cpool.tile([P, NBLK * H1], MBF16, tag="c_aldw", name="c_aldw")
            c_aldw2 = cpool.tile([P, NBLK], MBF16, tag="c_aldw2", name="c_aldw2")

            table1 = dram.tile([NPG, ROW1], MBF16)
            table2 = dram.tile([N, ROW2], MBF16, addr_space="Shared")
            ag_in = dram.tile([NL, ROW2], MBF16)

            h2 = bigpool.tile([P, NBLK * HID], MBF16, tag="h2")

            # ================= Layer 1 + layer-2 table =================
            with (
                tc.tile_pool(name="big1", bufs=1) as big1,
                tc.tile_pool(name="gath1", bufs=2) as gpool,
                tc.tile_pool(name="mbuf1", bufs=2) as mpool,
                tc.tile_pool(name="alph1", bufs=2) as apool,
            ):
                out1 = big1.tile([P, NBLK * F1], MBF16, tag="out1")

                with tc.tile_pool(name="psA", bufs=6, space="PSUM") as psA:
                    # Phase A: node table (replicated compute over all nodes)
                    stgs = []
                    for i in range(2):
                        s_ = stgA.tile(
                            [P, BT, ROW1], MBF16, tag=f"stgm_{i}", name=f"stgm_{i}"
                        )
                        nc.vector.memset(s_[:, :, AW1:ROW1], 0.0)
                        stgs.append(s_)
                    bi = 0
                    for b0 in range(0, NT_G, BT):
                        nb = min(BT, NT_G - b0)
                        xt = stgA.tile([P, BT * P], MBF16, tag="xt")
                        nc.sync.dma_start(
                            xt[:, 0 : nb * P], d_xT[:, b0 * P : (b0 + nb) * P]
                        )
                        stg = stgs[bi % 2]
                        bi += 1
                        for k in range(nb):
                            ps = psA.tile([P, AW1], FP32, tag="psA")
                            nc.tensor.matmul(
                                ps[:], xt[:, k * P : (k + 1) * P], c_rhs1[:],
                                start=True, stop=True,
                            )
                            if k % 2 == 0:
                                nc.scalar.activation(stg[:, k, 0:AW1], ps[:], ACTF.Copy)
                            else:
                                nc.vector.tensor_copy(stg[:, k, 0:AW1], ps[:])
                        nc.sync.dma_start(
                            table1[b0 * P : (b0 + nb) * P, :].rearrange(
                                "(k p) c -> p k c", p=P
                            ),
                            stg[:, 0:nb, :],
                        )

                    # per-window al_dst vectors from local x
                    for w in range(NBLK):
                        psd = psA.tile([P, H1], FP32, tag="psA")
                        nc.tensor.matmul(
                            psd[:], c_xloc[:, w * P : (w + 1) * P], c_wd1[:],
                            start=True, stop=True,
                        )
                        nc.scalar.activation(
                            c_aldw[:, w * H1 : (w + 1) * H1], psd[:], ACTF.Copy
                        )

                with (
                    tc.tile_pool(name="psAle1", bufs=2, space="PSUM") as psAle,
                    tc.tile_pool(name="psAgg1", bufs=3, space="PSUM") as psAgg,
                    tc.tile_pool(name="psS1", bufs=1, space="PSUM") as psS,
                ):
                    chunk_bufs = {}

                    def emit_chunk1(ch):
                        e0 = ch * CHT * P
                        eat = gpool.tile([ED, CHT * P], MBF16, tag="ea")
                        nc.sync.dma_start(eat[:], d_eaT[:, e0 : e0 + CHT * P])
                        mtt = gpool.tile([P, CHT * P], MBF16, tag="mtT")
                        nc.sync.dma_start(mtt[:], d_mtT[:, e0 : e0 + CHT * P])
                        mts = gpool.tile(
                            [P, CHT * P], MBF16, tag=f"mtE{ch % 2}"
                        )
                        nc.sync.dma_start(mts[:], d_mtE[:, e0 : e0 + CHT * P])
                        g1t = gpool.tile([P, CHT, ROW1], MBF16, tag=f"g1{ch % 2}")
                        nc.gpsimd.dma_gather(
                            g1t[:, :, :], table1[:, :],
                            c_srci[:, ch * P : (ch + 1) * P],
                            CHT * P, CHT * P, ROW1, single_packet=False,
                        )
                        pale = psAle.tile([P, CHT, H1], FP32, tag="pale")
                        for t in range(CHT):
                            w = win_of_tile[ch * CHT + t]
                            nc.tensor.matmul(
                                pale[:, t, :], eat[:, t * P : (t + 1) * P], c_Ae1[:],
                                start=True, stop=False,
                            )
                            nc.tensor.matmul(
                                pale[:, t, :], mtt[:, t * P : (t + 1) * P],
                                c_aldw[:, w * H1 : (w + 1) * H1],
                                start=False, stop=True,
                            )
                        alpha = apool.tile([P, CHT, H1], FP32, tag="alpha")
                        nc.vector.tensor_tensor(
                            alpha[:], g1t[:, :, F1 : F1 + H1], pale[:], ALU.add
                        )
                        lr = apool.tile([P, CHT, H1], FP32, tag="lr1")
                        nc.vector.tensor_scalar(
                            lr[:], alpha[:], 0.0, 1.0 - NEG, ALU.max, ALU.mult
                        )
                        nc.vector.scalar_tensor_tensor(
                            alpha[:], alpha[:], NEG, lr[:], ALU.mult, ALU.add
                        )
                        msgw = mpool.tile([P, CHT, F1 + H1], MBF16, tag="msgw")
                        nc.scalar.activation(msgw[:, :, F1 : F1 + H1], alpha[:], ACTF.Exp)
                        nc.vector.tensor_tensor(
                            msgw[:, :, 0:F1].rearrange("p t (h f) -> p t h f", f=HID),
                            g1t[:, :, 0:F1].rearrange("p t (h f) -> p t h f", f=HID),
                            msgw[:, :, F1 : F1 + H1]
                            .unsqueeze(3)
                            .broadcast_to((P, CHT, H1, HID)),
                            ALU.mult,
                        )
                        chunk_bufs[ch] = (msgw, mts)

                    psS1 = psS.tile([1, F1], FP32, tag="psS1")
                    psS2 = psS.tile([1, F1], FP32, tag="psS2")

                    def stats1(w):
                        nc.tensor.matmul(
                            psS1[:], c_onesb[:], out1[:, w * F1 : (w + 1) * F1],
                            start=(w == 0), stop=(w == NBLK - 1),
                        )
                        sqw = spool.tile([P, F1], MBF16, tag="sqw1")
                        nc.scalar.activation(
                            sqw[:], out1[:, w * F1 : (w + 1) * F1], ACTF.Square
                        )
                        nc.tensor.matmul(
                            psS2[:], c_onesb[:], sqw[:],
                            start=(w == 0), stop=(w == NBLK - 1),
                        )

                    t = 0
                    for w in range(NBLK):
                        psW = psAgg.tile([P, F1 + H1], FP32, tag="aggW")
                        for j in range(T_w[w]):
                            ch, tt = t // CHT, t % CHT
                            if tt == 0:
                                emit_chunk1(ch)
                            msgw, mts = chunk_bufs[ch]
                            nc.tensor.matmul(
                                psW[:], mts[:, tt * P : (tt + 1) * P], msgw[:, tt, :],
                                start=(j == 0), stop=(j == T_w[w] - 1),
                            )
                            t += 1
                        rden = spool.tile([P, H1], FP32, tag="rden1")
                        nc.vector.tensor_scalar(
                            rden[:], psW[:, F1 : F1 + H1], TINY, None, ALU.max
                        )
                        nc.vector.reciprocal(rden[:], rden[:])
                        nc.vector.tensor_tensor(
                            out1[:, w * F1 : (w + 1) * F1].rearrange(
                                "p (h f) -> p h f", f=HID
                            ),
                            psW[:, 0:F1].rearrange("p (h f) -> p h f", f=HID),
                            rden[:].unsqueeze(2).broadcast_to((P, H1, HID)),
                            ALU.mult,
                        )
                        if w >= 2:
                            stats1(w - 2)
                    stats1(NBLK - 2)
                    stats1(NBLK - 1)
                    if dbg:
                        nc.sync.dma_start(d_dbg1[:], out1[:])

                    # BN1 stats allreduce
                    bn1buf = spool.tile([1, 2 * F1], FP32, tag="bn1")
                    nc.vector.tensor_copy(bn1buf[:, 0:F1], psS1[:])
                    nc.vector.tensor_copy(bn1buf[:, F1 : 2 * F1], psS2[:])
                    bn1_in = dram.tile([1, 2 * F1], FP32)
                    bn1_out = dram.tile([1, 2 * F1], FP32, addr_space="Shared")
                    nc.sync.dma_start(bn1_in[:], bn1buf[:])
                    nc.gpsimd.collective_compute(
                        "AllReduce", ALU.add, replica_groups=rg,
                        ins=[bn1_in.opt()], outs=[bn1_out.opt()],
                    )
                    bnr1 = spool.tile([1, 2 * F1], FP32, tag="bn1r")
                    nc.sync.dma_start(bnr1[:], bn1_out[:])

                # BN1 row math; apply fused into transposed layout; layer-2 table
                with tc.tile_pool(name="psDE", bufs=2, space="PSUM") as psDE:
                    mean1 = spool.tile([1, F1], FP32, tag="mean1")
                    nc.scalar.activation(
                        mean1[:], bnr1[:, 0:F1], ACTF.Copy, scale=1.0 / N
                    )
                    var1 = spool.tile([1, F1], FP32, tag="var1")
                    nc.scalar.activation(
                        var1[:], bnr1[:, F1 : 2 * F1], ACTF.Copy, scale=1.0 / N
                    )
                    msq1 = spool.tile([1, F1], FP32, tag="msq1")
                    nc.vector.tensor_tensor(msq1[:], mean1[:], mean1[:], ALU.mult)
                    nc.vector.tensor_tensor(var1[:], var1[:], msq1[:], ALU.subtract)
                    nc.vector.tensor_scalar(var1[:], var1[:], EPS, None, ALU.add)
                    std1 = spool.tile([1, F1], FP32, tag="std1")
                    nc.scalar.activation(std1[:], var1[:], ACTF.Sqrt)
                    nc.vector.reciprocal(std1[:], std1[:])
                    scl1r = spool.tile([1, F1], FP32, tag="scl1r")
                    nc.vector.tensor_tensor(scl1r[:], c_g1[:], std1[:], ALU.mult)
                    sht1r = spool.tile([1, F1], FP32, tag="sht1r")
                    nc.vector.tensor_tensor(sht1r[:], mean1[:], scl1r[:], ALU.mult)
                    nc.vector.tensor_tensor(sht1r[:], c_b1[:], sht1r[:], ALU.subtract)
                    # transpose scale/shift rows into per-partition columns
                    c_w2b = spool.tile([P, 2, AW2], MBF16, tag="c_w2b")
                    nc.vector.tensor_copy(c_w2b[:], c_w2[:])
                    sclT = spool.tile([P, 2], FP32, tag="sclT")
                    shtT = spool.tile([P, 2], FP32, tag="shtT")
                    for cc in range(2):
                        pt1 = psDE.tile([P, 2], FP32, tag="pt1")
                        nc.tensor.matmul(
                            pt1[:, 0:1], scl1r[:, cc * P : (cc + 1) * P],
                            c_ones1[:, 0:1], start=True, stop=True,
                        )
                        nc.tensor.matmul(
                            pt1[:, 1:2], sht1r[:, cc * P : (cc + 1) * P],
                            c_ones1[:, 0:1], start=True, stop=True,
                        )
                        nc.vector.tensor_copy(sclT[:, cc : cc + 1], pt1[:, 0:1])
                        nc.vector.tensor_copy(shtT[:, cc : cc + 1], pt1[:, 1:2])

                    # table2 row: [xs2 (0:64) | ald2 (64) | als2 (65) | 1.0 (66) | 0...]
                    stg2s = []
                    for i in range(3):
                        s_ = stgA.tile(
                            [P, ROW2], MBF16, tag=f"stg2m_{i}", name=f"stg2m_{i}"
                        )
                        nc.vector.memset(s_[:, AW2 + 1 : ROW2], 0.0)
                        nc.vector.memset(s_[:, AW2 : AW2 + 1], 1.0)
                        stg2s.append(s_)
                    for w in range(NBLK):
                        hTw = spool.tile([P, 2, P], MBF16, tag="hTw")
                        for cc in range(2):
                            psT = psDE.tile([P, P], MBF16, tag="psT")
                            nc.tensor.transpose(
                                psT[:],
                                out1[:, w * F1 + cc * P : w * F1 + (cc + 1) * P],
                                c_identb[:],
                            )
                            nc.scalar.activation(
                                hTw[:, cc, :], psT[:], ACTF.Relu,
                                bias=shtT[:, cc : cc + 1], scale=sclT[:, cc : cc + 1],
                            )
                        psX = psDE.tile([P, AW2], FP32, tag="psX")
                        nc.tensor.matmul(
                            psX[:], hTw[:, 0, :], c_w2b[:, 0, :],
                            start=True, stop=False,
                        )
                        nc.tensor.matmul(
                            psX[:], hTw[:, 1, :], c_w2b[:, 1, :],
                            start=False, stop=True,
                        )
                        stg2 = stg2s[w % 3]
                        nc.vector.tensor_copy(stg2[:, 0:AW2], psX[:, 0:AW2])
                        nc.vector.tensor_copy(
                            c_aldw2[:, w : w + 1], psX[:, HID : HID + 1]
                        )
                        r0, r1 = w * P, min(NL, (w + 1) * P)
                        if r1 > r0:
                            nc.sync.dma_start(ag_in[r0:r1, :], stg2[0 : r1 - r0, :])
                    nc.gpsimd.collective_compute(
                        "AllGather", ALU.bypass, replica_groups=rg,
                        ins=[ag_in.opt()], outs=[table2.opt()],
                    )

            # ================= Phase F: layer-2 edges ======================
            # table2 row: [xs2 (0:64) | ald2 (64) | als2 (65) | 1.0 (66) | 0...]
            with (
                tc.tile_pool(name="gath2", bufs=2) as gpool2,
                tc.tile_pool(name="alph2", bufs=2) as apool2,
                tc.tile_pool(name="psAle2", bufs=2, space="PSUM") as psAle2,
                tc.tile_pool(name="psAgg2", bufs=4, space="PSUM") as psAgg2,
                tc.tile_pool(name="psBn2", bufs=1, space="PSUM") as psBn2,
            ):
                ps_bn2a = psBn2.tile([1, HID], FP32, tag="psbn2a")
                ps_bn2b = psBn2.tile([1, HID], FP32, tag="psbn2b")
                chunk2_bufs = {}
                AWD = AW2 + 1  # 67: [xs2 | ald2 | als2 | 1.0]

                def emit_chunk2(ch):
                    e0 = ch * CHT * P
                    eat = gpool2.tile([ED, CHT * P], MBF16, tag="ea2")
                    nc.sync.dma_start(eat[:], d_eaT[:, e0 : e0 + CHT * P])
                    mtt = gpool2.tile([P, CHT * P], MBF16, tag="mtT2")
                    nc.sync.dma_start(mtt[:], d_mtT[:, e0 : e0 + CHT * P])
                    mts = gpool2.tile(
                        [P, CHT * P], MBF16, tag=f"mtE2{ch % 3}"
                    )
                    nc.sync.dma_start(mts[:], d_mtE[:, e0 : e0 + CHT * P])
                    g3t = gpool2.tile([P, CHT, ROW2], MBF16, tag=f"g3{ch % 3}")
                    nc.gpsimd.dma_gather(
                        g3t[:, :, :], table2[:, :],
                        c_srci[:, ch * P : (ch + 1) * P],
                        CHT * P, CHT * P, ROW2, single_packet=False,
                    )
                    pale = psAle2.tile([P, CHT, 1], FP32, tag="pale2")
                    for t in range(CHT):
                        w = win_of_tile[ch * CHT + t]
                        nc.tensor.matmul(
                            pale[:, t, :], eat[:, t * P : (t + 1) * P], c_Ae2[:],
                            start=True, stop=False,
                        )
                        nc.tensor.matmul(
                            pale[:, t, :], mtt[:, t * P : (t + 1) * P],
                            c_aldw2[:, w : w + 1],
                            start=False, stop=True,
                        )
                    alpha = apool2.tile([P, CHT, 1], FP32, tag="alpha2")
                    nc.vector.tensor_tensor(
                        alpha[:], g3t[:, :, HID + 1 : HID + 2], pale[:], ALU.add
                    )
                    lr = apool2.tile([P, CHT, 1], FP32, tag="lr2")
                    nc.vector.tensor_scalar(
                        lr[:], alpha[:], 0.0, 1.0 - NEG, ALU.max, ALU.mult
                    )
                    nc.vector.scalar_tensor_tensor(
                        alpha[:], alpha[:], NEG, lr[:], ALU.mult, ALU.add
                    )
                    exb = apool2.tile([P, CHT, 1], FP32, tag="exb2")
                    nc.scalar.activation(exb[:], alpha[:], ACTF.Exp)
                    for t in range(CHT):
                        nc.vector.tensor_scalar(
                            g3t[:, t, 0:AWD], g3t[:, t, 0:AWD],
                            exb[:, t, :], None, ALU.mult,
                        )
                    chunk2_bufs[ch] = (g3t, mts)

                def stats2(w):
                    nc.tensor.matmul(
                        ps_bn2a[:], c_onesb[:], h2[:, w * HID : (w + 1) * HID],
                        start=(w == 0), stop=(w == NBLK - 1),
                    )
                    sqw = spool.tile([P, HID], MBF16, tag="sqw")
                    nc.scalar.activation(
                        sqw[:], h2[:, w * HID : (w + 1) * HID], ACTF.Square
                    )
                    nc.tensor.matmul(
                        ps_bn2b[:], c_onesb[:], sqw[:],
                        start=(w == 0), stop=(w == NBLK - 1),
                    )

                t = 0
                for w in range(NBLK):
                    pso = psAgg2.tile([P, AWD], FP32, tag="agg2")
                    for j in range(T_w[w]):
                        ch, tt = t // CHT, t % CHT
                        if tt == 0:
                            emit_chunk2(ch)
                        g3t, mts = chunk2_bufs[ch]
                        nc.tensor.matmul(
                            pso[:], mts[:, tt * P : (tt + 1) * P],
                            g3t[:, tt, 0:AWD],
                            start=(j == 0), stop=(j == T_w[w] - 1),
                        )
                        t += 1
                    rden2 = spool.tile([P, 1], FP32, tag="rden2")
                    nc.vector.tensor_scalar(
                        rden2[:], pso[:, AWD - 1 : AWD], TINY, None, ALU.max
                    )
                    nc.vector.reciprocal(rden2[:], rden2[:])
                    nc.scalar.activation(
                        h2[:, w * HID : (w + 1) * HID], pso[:, 0:HID], ACTF.Copy,
                        scale=rden2[:],
                    )
                    if w >= 2:
                        stats2(w - 2)
                stats2(NBLK - 2)
                stats2(NBLK - 1)

                if dbg:
                    nc.sync.dma_start(d_dbgh2[:], h2[:])
                bn2buf = spool.tile([1, 2 * HID], FP32, tag="bn2")
                nc.vector.tensor_copy(bn2buf[:, 0:HID], ps_bn2a[:])
                nc.vector.tensor_copy(bn2buf[:, HID : 2 * HID], ps_bn2b[:])
                bn2_in = dram.tile([1, 2 * HID], FP32)
                bn2_out = dram.tile([1, 2 * HID], FP32, addr_space="Shared")
                nc.sync.dma_start(bn2_in[:], bn2buf[:])
                nc.gpsimd.collective_compute(
                    "AllReduce", ALU.add, replica_groups=rg,
                    ins=[bn2_in.opt()], outs=[bn2_out.opt()],
                )
                bnr2 = spool.tile([1, 2, HID], FP32, tag="bn2r")
                nc.sync.dma_start(bnr2[:].rearrange("p a b -> p (a b)"), bn2_out[:])

            # ================= Phase G: BN2 + ReLU + pool ==================
            with tc.tile_pool(name="psG", bufs=2, space="PSUM") as psG:
                mean2 = spool.tile([1, HID], FP32, tag="mean2")
                nc.scalar.activation(mean2[:], bnr2[:, 0, :], ACTF.Copy, scale=1.0 / N)
                var2 = spool.tile([1, HID], FP32, tag="var2")
                nc.scalar.activation(var2[:], bnr2[:, 1, :], ACTF.Copy, scale=1.0 / N)
                msq2 = spool.tile([1, HID], FP32, tag="msq2")
                nc.vector.tensor_tensor(msq2[:], mean2[:], mean2[:], ALU.mult)
                nc.vector.tensor_tensor(var2[:], var2[:], msq2[:], ALU.subtract)
                nc.vector.tensor_scalar(var2[:], var2[:], EPS, None, ALU.add)
                std2 = spool.tile([1, HID], FP32, tag="std2")
                nc.scalar.activation(std2[:], var2[:], ACTF.Sqrt)
                nc.vector.reciprocal(std2[:], std2[:])
                scl2r = spool.tile([1, HID], FP32, tag="scl2r")
                nc.vector.tensor_tensor(scl2r[:], c_g2[:], std2[:], ALU.mult)
                sht2r = spool.tile([1, HID], FP32, tag="sht2r")
                nc.vector.tensor_tensor(sht2r[:], mean2[:], scl2r[:], ALU.mult)
                nc.vector.tensor_tensor(sht2r[:], c_b2[:], sht2r[:], ALU.subtract)
                psb = psG.tile([P, 2, HID], FP32, tag="psb2")
                nc.tensor.matmul(
                    psb[:, 0, :], c_ones1[:], scl2r[:], start=True, stop=True
                )
                nc.tensor.matmul(
                    psb[:, 1, :], c_ones1[:], sht2r[:], start=True, stop=True
                )
                sclF = spool.tile([P, HID], FP32, tag="sclF")
                nc.vector.tensor_copy(sclF[:], psb[:, 0, :])
                shtF = spool.tile([P, HID], FP32, tag="shtF")
                nc.vector.tensor_copy(shtF[:], psb[:, 1, :])

                ps_num = psG.tile([GB, HID], FP32, tag="psnum")
                ps_den = psG.tile([GB, 1], FP32, tag="psden")
                for wp in range(0, NBLK, 2):
                    kk = min(2, NBLK - wp)
                    hb = spool.tile([P, 2, HID], FP32, tag="hb")
                    h2s = h2[:, wp * HID : (wp + kk) * HID].rearrange(
                        "p (k h) -> p k h", h=HID
                    )
                    nc.vector.tensor_tensor(
                        hb[:, 0:kk, :], h2s,
                        sclF[:].unsqueeze(1).broadcast_to((P, kk, HID)), ALU.mult
                    )
                    nc.vector.tensor_tensor(
                        hb[:, 0:kk, :], hb[:, 0:kk, :],
                        shtF[:].unsqueeze(1).broadcast_to((P, kk, HID)), ALU.add
                    )
                    nc.vector.tensor_scalar(
                        hb[:, 0:kk, :], hb[:, 0:kk, :], 0.0, None, ALU.max
                    )
                    gtmp = spool.tile([P, 2, HID], FP32, tag="gtmp")
                    nc.vector.tensor_tensor(
                        gtmp[:, 0:kk, :], hb[:, 0:kk, :],
                        c_WgF[:].unsqueeze(1).broadcast_to((P, kk, HID)), ALU.mult
                    )
                    gate = spool.tile([P, 2, 1], FP32, tag="gate")
                    nc.vector.reduce_sum(gate[:, 0:kk, :], gtmp[:, 0:kk, :], AX.X)
                    ex = spool.tile([P, 2, 1], FP32, tag="exg")
                    nc.scalar.activation(ex[:, 0:kk, :], gate[:, 0:kk, :], ACTF.Exp)
                    for k in range(kk):
                        w = wp + k
                        bex = spool.tile([P, GB], FP32, tag="bex")
                        nc.vector.tensor_scalar(
                            bex[:], c_iotaG[:], c_brel[:, w : w + 1], ex[:, k, :],
                            ALU.is_equal, ALU.mult,
                        )
                        nc.tensor.matmul(
                            ps_num[:], bex[:], hb[:, k, :],
                            start=(w == 0), stop=(w == NBLK - 1),
                        )
                        nc.tensor.matmul(
                            ps_den[:], bex[:], c_ones[:],
                            start=(w == 0), stop=(w == NBLK - 1),
                        )
                poolbuf = spool.tile([GB, HID + 1], FP32, tag="poolbuf")
                nc.vector.tensor_copy(poolbuf[:, 0:HID], ps_num[:])
                nc.vector.tensor_copy(poolbuf[:, HID : HID + 1], ps_den[:])
                pool_in = dram.tile([GB, HID + 1], FP32)
                pool_out = dram.tile([GB, HID + 1], FP32, addr_space="Shared")
                nc.sync.dma_start(pool_in[:], poolbuf[:])
                nc.gpsimd.collective_compute(
                    "AllReduce", ALU.add, replica_groups=rg,
                    ins=[pool_in.opt()], outs=[pool_out.opt()],
                )
                poolr = spool.tile([GB, HID + 1], FP32, tag="poolr")
                nc.sync.dma_start(poolr[:], pool_out[:])
                dinv = spool.tile([GB, 1], FP32, tag="dinv")
                nc.vector.reciprocal(dinv[:], poolr[:, HID : HID + 1])
                res = spool.tile([GB, HID], FP32, tag="res")
                nc.vector.tensor_scalar(
                    res[:], poolr[:, 0:HID], dinv[:], None, ALU.mult
                )
                nc.sync.dma_start(d_out[:], res[:])

    nc.compile()
    return nc


# ---------------------------------------------------------------- runner
def make_in_maps(meta, shared, percore):
    return [{**shared, **pc} for pc in percore]


def run(inputs, ncores=8, trace=False, sim=False, GB=64, dbg=False):
    meta, shared, percore = prep(inputs, ncores, GB=GB)
    nc = build(meta, dbg=dbg)
    in_maps = make_in_maps(meta, shared, percore)
    if sim:
        from concourse.bass_interp import MultiCoreSim

        msim = MultiCoreSim(nc, ncores)
        for c in range(ncores):
            for k, v in in_maps[c].items():
                msim.cores[c].tensor(k)[:] = v
        msim.simulate()
        return msim.cores[0].mem_tensor("out").copy(), (msim, meta)
    from concourse.bass_utils import run_bass_kernel_spmd

    res = run_bass_kernel_spmd(nc, in_maps, core_ids=list(range(ncores)), trace=trace)
    return res.results[0]["out"], res


# ---------------------------------------------------------------- kernel API
_CACHE = {}


def _run_full(inputs, trace=False):
    meta, shared, percore = prep(inputs, 8, GB=64)
    key = (meta["EPC"], meta["N"])
    if key not in _CACHE:
        _CACHE[key] = build(meta)
    nc = _CACHE[key]
    in_maps = make_in_maps(meta, shared, percore)
    from concourse.bass_utils import run_bass_kernel_spmd

    res = run_bass_kernel_spmd(nc, in_maps, core_ids=list(range(8)), trace=trace)
    return np.asarray(res.results[0]["out"], np.float32), res


def kernel(**inputs):
    out, _ = _run_full(inputs, trace=False)
    return out
